# revision 70
# baseline (speedup 1.0000x reference)
"""Trainium2 Bass kernel for a dense transformer block (self-attn + cross-attn + MLP).

Sharding: data-parallel over batch, one batch element per NeuronCore (B=8, 8 cores),
no collectives. Activations are feature-major ([C, T]) on chip; projections
consume weight tiles directly from DRAM.

Perf structure (baseline 620us -> 562us on the timeline model), built around
fp8e4m3 DoubleRow matmuls (0.5 cycles/row, two 128-deep K-tiles per
instruction = 4x bf16 MAC throughput):
- All big projections run fp8 DoubleRow. Weights are host-split into
  hi+lo e4m3 pairs (x64 scale); accuracy-critical matmuls (qkv, fc, mproj)
  use the 3-term product (Ahi+Alo)@Whi + Ahi@Wlo at 0.75x the bf16 cycle
  cost with near-f16 accuracy. Cheap paths (q2/k2/v2/cproj, scores, AV) run
  single-fp8. aproj stays f16 (its input would need a split to survive the
  concentrated-attention early tokens, and f16 at 1 cyc/row is cheaper).
- Activation tiles are fp8 c-pair tiles [P, 2, 512] (DoubleRow moving
  layout); LN outputs are produced as hi/lo pairs via an f16 intermediate.
- Scores run DoubleRow on 32-partition d-half pairs: the q/k weight columns
  are host-permuted so head a's 64 dims land on partitions 32a..32a+31 x2
  halves ([P, 2, T] q/k tiles).
- Softmax: S^T layout, probs in fp8 at x4 scale (exp tail < e4m3 max; the
  num/denom ratio cancels the scale and the prob noise on single-key rows);
  denominator from a ones column in the augmented V (0.125 in cross = x8 o2
  scale). One exp instruction covers each st-pair via a [P, 2, 512] psum.
- Bias algebra done on host: LN gain/bias folded into consuming weights,
  v-bias folded into aproj/cproj biases (prob-weighted average is affine),
  k-bias dropped entirely (softmax-invariant), mproj bias folded into
  cproj's, q/k stored at x8 with biases pre-scaled.
- Overlap: qkv for both token chunks is produced inside the tch0 attention
  window; cross-attn prep, aproj/ln1b/q2 chains, cproj, LN2 stats and the
  chunk-0 h2 hi/lo split all drain into later attention windows; fc reuses
  each weight slab for both chunks; transpose-out of chunk 0 runs between
  the two mproj passes.
- HW legality notes: f32r matmul inputs need a rounding producer (no DMA
  bitcasts); matmul operand dtypes cannot mix; gpsimd cannot touch PSUM and
  runs tensor ops at 0.42 impl efficiency (keep it to broadcasts/memsets).
"""

import math
import sys
import numpy as np

sys.path.insert(0, "/opt/trn_rl_repo")

B, T, C = 8, 1024, 1024
H = 16
D = C // H          # 64
TI = 256
FF = 4 * C          # 4096
EPS = 1e-5
NCT = C // 128      # 8 c tiles
NTT = T // 128      # 8 t tiles
NFT = FF // 128     # 32 ff tiles
P = 128

_CACHED = {}

WEIGHT_NAMES = ()  # no bf16 weights remain
FP8_SCALE = 64.0


def _build():
    import concourse.tile as tile
    from concourse import bacc, mybir
    from concourse.masks import make_identity

    F32, F32R, BF16 = mybir.dt.float32, mybir.dt.float32r, mybir.dt.bfloat16
    FP8, F16 = mybir.dt.float8e4, mybir.dt.float16
    AF = mybir.ActivationFunctionType
    OP = mybir.AluOpType
    DR = mybir.MatmulPerfMode.DoubleRow
    LOGPS = math.log(4.0)

    nc = bacc.Bacc("TRN2", target_bir_lowering=False, debug=False, num_devices=8)

    dr = {}
    dr["x"] = nc.dram_tensor("x", [T, C], F32, kind="ExternalInput")
    dr["x_img_feats"] = nc.dram_tensor("x_img_feats", [TI, C], F32, kind="ExternalInput")
    for nm, shp in [
        ("ln1_g", [C]), ("ln1_b", [C]), ("ln2_g", [C]), ("ln2_b", [C]),
        ("b_attn", [C]), ("b_aproj", [C]),
        ("bq", [C]), ("bcproj", [C]),
        ("b_fc", [FF]),
    ]:
        dr[nm] = nc.dram_tensor(nm, shp, F32, kind="ExternalInput")
    dr["W_aproj"] = nc.dram_tensor("W_aproj", [C, C], F16, kind="ExternalInput")
    for nm, shp in [
        ("W_attn_hi", [C, 3 * C]), ("W_attn_lo", [C, 3 * C]),
        ("Wq", [C, C]), ("Wk", [C, C]), ("Wv", [C, C]), ("Wcproj", [C, C]),
        ("W_fc_hi", [C, FF]), ("W_fc_lo", [C, FF]),
        ("W_mproj_hi", [FF, C]), ("W_mproj_lo", [FF, C]),
    ]:
        dr[nm] = nc.dram_tensor(nm, shp, FP8, kind="ExternalInput")
    out_d = nc.dram_tensor("out", [T, C], F32, kind="ExternalOutput")

    def W2d(name):
        return dr[name].ap()

    with tile.TileContext(nc) as tc, nc.allow_low_precision(
        reason="bf16 weights/activations are within the 2e-2 tolerance"
    ):
        kw_cms = []

        def openp(**kw):
            cm = tc.tile_pool(**kw)
            return cm, cm.__enter__()

        def openkw(**kw):
            cm, p = openp(**kw)
            kw_cms.append(cm)
            return p

        # ---------------- kernel-wide pools (left-stack base) ----------------
        constp = openkw(name="const", bufs=1)
        fsrp = openkw(name="fsr", bufs=2)       # f32r [128,512] squares
        abp = openkw(name="ab", bufs=1)         # A_b/B_b [128,1024]
        rowp = openkw(name="rows", bufs=4)      # one [1,1024] "row" tag
        rbp = openkw(name="rb", bufs=3)         # [64,512] + [1,512] rden
        tmpp = openkw(name="tmp", bufs=6)       # [128,512] f16 split scratch

        # ---------------- constants ----------------
        ones_col = constp.tile([P, 32], FP8)
        nc.vector.memset(ones_col, 1.0)
        ones_f = constp.tile([P, 1], F32)
        nc.vector.memset(ones_f, 1.0)
        ones128R = constp.tile([P, 1], F32R)
        nc.vector.tensor_copy(out=ones128R, in_=ones_f)
        eps_t = constp.tile([1, 1], F32)
        nc.vector.memset(eps_t, EPS)
        logps_t = constp.tile([P, 1], F32)
        nc.vector.memset(logps_t, LOGPS)

        # diagonal causal mask [128, 128]: keep (1.0) iff col >= row.
        master = constp.tile([P, P], FP8)
        nc.gpsimd.memset(master, 1.0)
        nc.gpsimd.affine_select(
            out=master, in_=master, compare_op=OP.is_ge, fill=0.0,
            base=0, pattern=[[1, P]], channel_multiplier=-1)
        # odd-half pair mask [128, 256]: zeros then the diagonal mask, for the
        # below-diagonal block + diagonal tile of an st-pair's odd half.
        mask256 = constp.tile([P, 2 * P], FP8)
        nc.gpsimd.memset(mask256[:, 0:P], 0.0)
        nc.gpsimd.tensor_copy(out=mask256[:, P:2 * P], in_=master)

        # bias/gain columns: contiguous [nf,128] row DMA + one PE transpose
        psB_cm, psB = openp(name="psB", bufs=2, space="PSUM")
        stg_cm, stgp = openp(name="bstage", bufs=3)
        idf = constp.tile([P, P], F32)
        make_identity(nc, idf)
        identR = constp.tile([P, P], F32R)
        nc.gpsimd.tensor_copy(out=identR, in_=idf)

        def load_cols(name, nf, lo=0):
            row = stgp.tile([1, nf * P], F32, tag=name + "_s", name=name + "_s",
                            bufs=1)
            nc.sync.dma_start(
                out=row,
                in_=dr[name].ap()[lo:lo + nf * P].rearrange("(a c) -> a c", a=1))
            tps = psB.tile([P, nf], F32, tag="bt", name="bt")
            for i in range(nf):
                nc.tensor.transpose(tps[:, i:i + 1], row[0:1, i * P:(i + 1) * P],
                                    idf[0:1, 0:1])
            t = constp.tile([P, nf], F32, name=name + "_c")
            nc.vector.tensor_copy(out=t, in_=tps)
            return t

        bqk = load_cols("b_attn", 8)
        bqk64 = constp.tile([P, 8], F32, name="bqk64")
        nc.vector.tensor_scalar_mul(out=bqk64, in0=bqk, scalar1=8.0)
        bq_c = load_cols("bq", NCT)
        bap_c = load_cols("b_aproj", NCT)
        bcp_c = load_cols("bcproj", NCT)
        bfc_c = load_cols("b_fc", NFT)
        stg_cm.__exit__(None, None, None)
        psB_cm.__exit__(None, None, None)

        # ---------------- helpers ----------------
        def bcast_row_bf16(src_ap, dest_pool, tag):
            """bias vector in DRAM -> [128, C] bf16 broadcast tile, via two
            [1,512] row chunks (keeps the rows pool slot at 2KB)."""
            dest = dest_pool.tile([P, C], BF16, tag=tag, name=tag)
            for half in range(2):
                row = rowp.tile([1, 512], F32, tag="row", name="brow_h")
                nc.sync.dma_start(
                    out=row,
                    in_=src_ap[512 * half:512 * (half + 1)]
                    .rearrange("(a c) -> a c", a=1))
                rowb = rowp.tile([1, 512], BF16, tag="row", name="rowb_h")
                nc.vector.tensor_copy(out=rowb, in_=row)
                nc.gpsimd.partition_broadcast(
                    dest[:, 512 * half:512 * (half + 1)].bitcast(F32),
                    rowb.bitcast(F32))
            return dest

        def load_wpair(wap, co2, wpool, dtype=BF16):
            """W[:, co2*256:(co2+1)*256] -> [128, K/128, 256] slab."""
            nk = wap.shape[0] // P
            wr = wpool.tile([P, nk, 256], dtype, tag="ws", name="ws")
            nc.sync.dma_start(
                out=wr,
                in_=wap[:, co2 * 256:(co2 + 1) * 256]
                .rearrange("(c p) f -> p c f", p=P))
            return wr

        def ln_stats_tch(xtiles, psp, tch, A_b, B_b, tag="s"):
            sl = slice(512 * tch, 512 * (tch + 1))
            st2 = psp.tile([P, 2, 512], F32, tag=tag, name="lnst")
            sp = st2[0:1, 0, :]
            qp = st2[0:1, 1, :]
            for c in range(NCT):
                xs = xtiles[c][:, sl]
                nc.tensor.matmul(sp, ones128R, xs, start=(c == 0), stop=(c == NCT - 1))
                sq = fsrp.tile([P, 512], F32R, tag="sq", name="sq")
                if c % 2:
                    nc.scalar.activation(out=sq, in_=xs, func=AF.Square, scale=1.0)
                else:
                    nc.vector.tensor_tensor(out=sq, in0=xs, in1=xs, op=OP.mult)
                nc.tensor.matmul(qp, ones128R, sq, start=(c == 0), stop=(c == NCT - 1))
            mu = rowp.tile([1, 512], F32, tag="row", name="mu")
            msq = rowp.tile([1, 512], F32, tag="row", name="msq")
            nc.vector.tensor_scalar_mul(out=mu, in0=sp, scalar1=1.0 / C)
            nc.vector.tensor_scalar_mul(out=msq, in0=qp, scalar1=1.0 / C)
            musq = rowp.tile([1, 512], F32, tag="row", name="musq")
            nc.vector.tensor_tensor(out=musq, in0=mu, in1=mu, op=OP.mult)
            nc.vector.tensor_tensor(out=msq, in0=msq, in1=musq, op=OP.subtract)
            nc.scalar.activation(out=musq, in_=msq, func=AF.Sqrt, bias=eps_t, scale=1.0)
            arow = rowp.tile([1, 512], BF16, tag="row", name="arow")
            nc.vector.reciprocal(out=arow, in_=musq)
            brow = rowp.tile([1, 512], BF16, tag="row", name="brow")
            nc.vector.scalar_tensor_tensor(out=brow, in0=mu, scalar=-1.0, in1=arow,
                                           op0=OP.mult, op1=OP.mult)
            nc.gpsimd.partition_broadcast(A_b[:, sl].bitcast(F32), arow.bitcast(F32))
            nc.gpsimd.partition_broadcast(B_b[:, sl].bitcast(F32), brow.bitcast(F32))

        def ln_ab():
            A_b = abp.tile([P, T], BF16, tag="A_b", name="A_b")
            B_b = abp.tile([P, T], BF16, tag="B_b", name="B_b")
            return A_b, B_b

        def ln_apply(xtiles, A_b, B_b, hpool, tsl=slice(0, T), pool_add=True,
                     pool_mult=False):
            # ht = x*A_b + B_b (LN gain/bias are folded into the consuming
            # weights host-side). Two plain tensor_tensor ops, in place in the
            # bf16 h tile; engine placement follows each window's slack.
            w = tsl.stop - tsl.start
            htiles = []
            for c in range(NCT):
                e1 = nc.gpsimd if pool_mult else nc.vector
                e2 = nc.gpsimd if pool_add and c % 2 == 0 else nc.vector
                ht = hpool.tile([P, w], BF16, tag="h", name="h")
                e1.tensor_tensor(out=ht, in0=xtiles[c][:, tsl], in1=A_b[:, tsl],
                                 op=OP.mult)
                e2.tensor_tensor(out=ht, in0=ht, in1=B_b[:, tsl], op=OP.add)
                htiles.append(ht)
            return htiles

        def ln_apply_split(A_b, B_b, hpool, tsl):
            """h = xT*A + B -> fp8 hi/lo c-pair tiles [P, 2, 512] (hi+lo ~= h
            to f16 accuracy, for 3-term DoubleRow matmuls). Copies ride the
            Act engine; Pool (0.42 impl efficiency) only gets every other
            mult so DVE stays below the window budget."""
            w = tsl.stop - tsl.start
            his, los = [], []
            for c2 in range(NCT // 2):
                hi = hpool.tile([P, 2, w], FP8, tag="hhi", name="hhi")
                lo = hpool.tile([P, 2, w], FP8, tag="hlo", name="hlo")
                for half in range(2):
                    c = 2 * c2 + half
                    tmp = tmpp.tile([P, w], F16, tag="htmp", name="htmp")
                    e1 = nc.vector if c % 2 else nc.gpsimd
                    e1.tensor_tensor(out=tmp, in0=xT[c][:, tsl], in1=A_b[:, tsl],
                                     op=OP.mult)
                    nc.vector.tensor_tensor(out=tmp, in0=tmp, in1=B_b[:, tsl],
                                            op=OP.add)
                    nc.scalar.copy(out=hi[:, half, :], in_=tmp)
                    e4 = nc.gpsimd if c % 2 else nc.vector
                    e4.tensor_tensor(out=lo[:, half, :], in0=tmp,
                                     in1=hi[:, half, :], op=OP.subtract)
                his.append(hi)
                los.append(lo)
            return his, los

        def ln_apply_pair(A_b, B_b, hpool, tsl):
            """h = xT*A + B -> single fp8 c-pair tiles [P, 2, 512] (f16
            intermediate keeps the pre-round value accurate)."""
            w = tsl.stop - tsl.start
            tiles = []
            for c2 in range(NCT // 2):
                ht2 = hpool.tile([P, 2, w], FP8, tag="h", name="h")
                for half in range(2):
                    c = 2 * c2 + half
                    tmp = tmpp.tile([P, w], F16, tag="htmp", name="htmp")
                    e1 = nc.vector if c % 2 else nc.gpsimd
                    e1.tensor_tensor(out=tmp, in0=xT[c][:, tsl], in1=A_b[:, tsl],
                                     op=OP.mult)
                    e2 = nc.gpsimd if c % 2 else nc.vector
                    e2.tensor_tensor(out=ht2[:, half, :], in0=tmp,
                                     in1=B_b[:, tsl], op=OP.add)
                tiles.append(ht2)
            return tiles

        def attn_chunk(kq_of, va_terms, npair, oview, h, tch, psp, ppool, causal):
            """fp8 attention for one head x 512-query chunk. Probs land in
            st-pair tiles [P, 2, 512] so AV runs fp8 DoubleRow; va_terms is
            (va_hi, va_lo) for split v, or (va_hi,) for single."""
            (kt, ko), (qt, qo) = kq_of(h)
            base = 512 * tch
            ops = psp.tile([65, 512], F32, tag="o", name="o")
            pts = []
            for jp in range(npair):
                st0 = 2 * jp
                poff = max(0, P * st0 - base) if causal else 0
                pt = ppool.tile([P, 2, 512], FP8, tag="p", name="p")
                sp2 = psp.tile([P, 2, 512], F32, tag="s", name="s")
                for i in range(2):
                    st = st0 + i
                    nc.tensor.matmul(sp2[:, i, poff:512],
                                     kt[ko:ko + 32, :, st * P:(st + 1) * P],
                                     qt[qo:qo + 32, :, base + poff:base + 512],
                                     start=True, stop=True, tile_position=(ko, 0),
                                     perf_mode=DR)
                # q,k are stored x8 (psum x64); probs come out x4: the
                # unnormalized exp tail (~e^3) stays under e4m3's 240 max;
                # single-key rows cancel prob noise in the num/denom ratio.
                # One exp covers both halves of the pair (banks are adjacent).
                nc.scalar.activation(out=pt[:, :, poff:512],
                                     in_=sp2[:, :, poff:512], func=AF.Exp,
                                     scale=0.125 / FP8_SCALE, bias=logps_t)
                if causal:
                    dpos = P * st0 - base
                    if dpos >= 0:
                        nc.vector.tensor_tensor(out=pt[:, 0, dpos:dpos + P],
                                                in0=pt[:, 0, dpos:dpos + P],
                                                in1=master, op=OP.mult)
                        nc.vector.tensor_tensor(out=pt[:, 1, dpos:dpos + 2 * P],
                                                in0=pt[:, 1, dpos:dpos + 2 * P],
                                                in1=mask256, op=OP.mult)
                pts.append((pt, poff))
            nterm = len(va_terms)
            for ti, va in enumerate(va_terms):
                for jp in range(npair):
                    pt, poff = pts[jp]
                    nc.tensor.matmul(ops[:, poff:512],
                                     va[jp][:, :, 65 * h:65 * h + 65],
                                     pt[:, :, poff:512],
                                     start=(ti == 0 and jp == 0),
                                     stop=(ti == nterm - 1 and jp == npair - 1),
                                     perf_mode=DR, skip_group_check=True)
            rden = rbp.tile([1, 512], BF16, tag="rden", name="rden")
            nc.vector.reciprocal(out=rden, in_=ops[64:65, :])
            rb = rbp.tile([64, 512], BF16, tag="rb", name="rb")
            # broadcast is a byte copy: bitcast bf16 pairs to f32 to halve
            # the per-element ucode cost
            nc.gpsimd.partition_broadcast(rb.bitcast(F32), rden.bitcast(F32))
            nc.vector.tensor_tensor(out=oview(h, base), in0=ops[0:64, :],
                                    in1=rb, op=OP.mult)

        # ========== right stack: xT doubles as the residual stream ==========
        xT_cm, xTp = openp(name="xT", bufs=NCT, side="right")
        xT = [xTp.tile([P, T], F32R, tag="xT", name="xT") for _ in range(NCT)]

        # imgT lives (left) from P0 until cross attention ends; c-pair fp8
        # tiles [P, 2, TI] feed the k2/v2 DoubleRow matmuls directly
        img_cm, imgp = openp(name="img", bufs=NCT // 2)
        imgT = [imgp.tile([P, 2, TI], FP8, tag="imgT", name="imgT")
                for _ in range(NCT // 2)]

        # ================= P0: load & transpose x and img =================
        h1_cm, hp = openp(name="h1", bufs=2 * NCT)
        tok_cm, tokp = openp(name="tok0", bufs=4)
        tp_cm, tpp = openp(name="psT0", bufs=4, space="PSUM")
        ps0_cm, ps0p = openp(name="psT0s", bufs=1, space="PSUM")

        def transpose_pair(src_ap, dst_view, tt0):
            # two token tiles staged into one PSUM bank per feature chunk:
            # one 256-wide copy instead of two 128-wide ones
            toks = []
            for tt in (tt0, tt0 + 1):
                tok = tokp.tile([P, C], F32, tag="tok", name="tok")
                nc.sync.dma_start(out=tok, in_=src_ap[tt * P:(tt + 1) * P, :])
                toks.append(tok)
            for c in range(NCT):
                tp2 = tpp.tile([P, 512], F32, tag="tp", name="tp")
                for i, tok in enumerate(toks):
                    nc.tensor.transpose(tp2[:, i * P:(i + 1) * P],
                                        tok[:, c * P:(c + 1) * P], idf)
                sl = slice(tt0 * P, (tt0 + 2) * P)
                if c % 2:
                    nc.scalar.copy(out=dst_view(c, sl), in_=tp2[:, 0:2 * P])
                else:
                    nc.vector.tensor_copy(out=dst_view(c, sl), in_=tp2[:, 0:2 * P])

        # matmul LN1 stats per token chunk, pipelined against the second
        # half of the input transpose and the h-split vector work
        A_b, B_b = ln_ab()
        hhi_t, hlo_t = [None, None], [None, None]
        transpose_pair(dr["x"].ap(), lambda c, sl: xT[c][:, sl], 0)
        transpose_pair(dr["x"].ap(), lambda c, sl: xT[c][:, sl], 2)
        ln_stats_tch(xT, ps0p, 0, A_b, B_b, tag="lnst")
        transpose_pair(dr["x"].ap(), lambda c, sl: xT[c][:, sl], 4)
        hhi_t[0], hlo_t[0] = ln_apply_split(A_b, B_b, hp, slice(0, 512))
        transpose_pair(dr["x"].ap(), lambda c, sl: xT[c][:, sl], 6)
        ln_stats_tch(xT, ps0p, 1, A_b, B_b, tag="lnst")
        transpose_pair(dr["x_img_feats"].ap(),
                       lambda c, sl: imgT[c // 2][:, c % 2, sl], 0)
        hhi_t[1], hlo_t[1] = ln_apply_split(A_b, B_b, hp, slice(512, 1024))
        ps0_cm.__exit__(None, None, None)
        tp_cm.__exit__(None, None, None)
        tok_cm.__exit__(None, None, None)

        # ====== merged qkv + self-attention (+ hoisted cross-attn prep) ======
        # va tiles hold st-PAIRS: va_hi/va_lo[jp] = [P, 2, 16*65] fp8 so the
        # AV matmul runs fp8 DoubleRow over two key tiles per instruction.
        # v is hi+lo split (2-term AV); the ones column lives in hi only.
        vap_cm, vap = openp(name="vaug", bufs=NTT, side="right")
        va_hi = [vap.tile([P, 2, 16 * 65], FP8, tag="va", name="va")
                 for _ in range(NTT // 2)]
        va_lo = [vap.tile([P, 2, 16 * 65], FP8, tag="va", name="va")
                 for _ in range(NTT // 2)]
        qk_cm, qkp = openp(name="qk", bufs=8, side="right")
        # [P, 2, T]: partitions 32a..32a+31 hold head a's d-halves (dim1);
        # host permutes W's q/k columns to this layout so the score matmul
        # runs fp8 DoubleRow on 32-partition tiles
        qk_t = [qkp.tile([P, 2, T], FP8, tag="qk", name="qk") for _ in range(8)]

        k2_cm, k2p = openp(name="k2", bufs=NCT // 2)
        v2_cm, v2p = openp(name="v2", bufs=2)
        hb_cm, hbp = openp(name="hb", bufs=NCT)
        q2_cm, q2p = openp(name="q2", bufs=NCT // 2)
        w23_cm, w23 = openp(name="w23", bufs=4)
        psAC_cm, accp = openp(name="psAC", bufs=2, space="PSUM")

        o_cm, opool = openp(name="o1", bufs=NCT)
        pp_cm, pp = openp(name="pp1", bufs=7)
        psS_cm, psS = openp(name="psS1", bufs=2, space="PSUM")

        otiles = [opool.tile([P, T], F16, tag="ot", name="ot") for _ in range(NCT)]
        k2_t = [k2p.tile([P, 2, TI], FP8, tag="k2", name="k2")
                for _ in range(NCT // 2)]
        v2aug = v2p.tile([P, 2, 16 * 65], FP8, tag="va2", name="va2")
        q2_t = [q2p.tile([P, 2, T], FP8, tag="q2", name="q2")
                for _ in range(NCT // 2)]

        def kq_self(h):
            return (qk_t[4 + h // 4], 32 * (h % 4)), (qk_t[h // 4], 32 * (h % 4))

        def kq_cross(h):
            return (k2_t[h // 4], 32 * (h % 4)), (q2_t[h // 4], 32 * (h % 4))

        side = []

        def drain(n=1):
            for _ in range(n):
                if side:
                    side.pop(0)()

        # ones columns: 1.0 in va_hi (softmax denominator), 0 in va_lo,
        # 0.125 in v2aug (bakes the x8 fp8 scaling of o2 into the ratio)
        for jp in range(NTT // 2):
            nc.gpsimd.tensor_copy(
                out=va_hi[jp].rearrange("p two (h x) -> p two h x", x=65)[:, :, :, 64:65],
                in_=ones_col.rearrange("p (two h x) -> p two h x", x=1, two=2))
            nc.gpsimd.memset(
                va_lo[jp].rearrange("p two (h x) -> p two h x", x=65)[:, :, :, 64:65],
                0.0)
        nc.gpsimd.memset(
            v2aug.rearrange("p two (h x) -> p two h x", x=65)[:, :, :, 64:65], 0.125)

        def v_group(cc):
            # one W slab load feeds all 8 token tiles (both tch)
            whi = load_wpair(dr["W_attn_hi"].ap(), 8 + cc, w23, FP8)
            wlo = load_wpair(dr["W_attn_lo"].ap(), 8 + cc, w23, FP8)
            for tt in range(NTT):
                vtch = tt // 4
                vps = accp.tile([P, 512], F32, tag="acc", name="acc")[:, 0:256]
                n = 0
                for wsl, hts in ((whi, hhi_t[vtch]), (whi, hlo_t[vtch]),
                                 (wlo, hhi_t[vtch])):
                    for c2 in range(NCT // 2):
                        nc.tensor.matmul(
                            vps, hts[c2][:, :, (tt % 4) * P:(tt % 4 + 1) * P],
                            wsl[:, 2 * c2:2 * c2 + 2, :],
                            start=(n == 0), stop=(n == 11), perf_mode=DR)
                        n += 1
                vr = vps.rearrange("p (h x) -> p h x", x=64)
                hi_dst = va_hi[tt // 2].rearrange(
                    "p two (h x) -> p two h x", x=65)[:, tt % 2, 4 * cc:4 * (cc + 1), 0:64]
                lo_dst = va_lo[tt // 2].rearrange(
                    "p two (h x) -> p two h x", x=65)[:, tt % 2, 4 * cc:4 * (cc + 1), 0:64]
                # v bias is folded into b_aproj host-side (AV is an affine
                # average: sum_p(v + b) = AV + b), so hi is a pure scaled copy
                nc.scalar.activation(out=hi_dst, in_=vr, func=AF.Identity,
                                     scale=1.0 / FP8_SCALE)
                nc.vector.scalar_tensor_tensor(out=lo_dst, in0=vr,
                                               scalar=1.0 / FP8_SCALE,
                                               in1=hi_dst, op0=OP.mult,
                                               op1=OP.subtract)

        def qk_pair(p_idx, dst0, bias0):
            # one W slab load produces both token chunks of these q/k columns
            whi = load_wpair(dr["W_attn_hi"].ap(), p_idx, w23, FP8)
            wlo = load_wpair(dr["W_attn_lo"].ap(), p_idx, w23, FP8)
            for fh in range(2):
                for tch in range(2):
                    aps = accp.tile([P, 512], F32, tag="acc", name="acc")
                    n = 0
                    for wsl, hts in ((whi, hhi_t[tch]), (whi, hlo_t[tch]),
                                     (wlo, hhi_t[tch])):
                        for c2 in range(NCT // 2):
                            nc.tensor.matmul(aps,
                                             wsl[:, 2 * c2:2 * c2 + 2,
                                                 128 * fh:128 * (fh + 1)],
                                             hts[c2], start=(n == 0),
                                             stop=(n == 11), perf_mode=DR)
                            n += 1
                    # q/k stored at x8 scale; k bias dropped (softmax-invariant)
                    if p_idx < 4:
                        nc.vector.tensor_scalar(
                            out=qk_t[dst0][:, fh, 512 * tch:512 * (tch + 1)],
                            in0=aps,
                            scalar1=bqk64[:, bias0 + fh:bias0 + fh + 1],
                            scalar2=8.0 / FP8_SCALE, op0=OP.add, op1=OP.mult)
                    else:
                        nc.scalar.activation(
                            out=qk_t[dst0][:, fh, 512 * tch:512 * (tch + 1)],
                            in_=aps, func=AF.Identity,
                            scale=8.0 / FP8_SCALE)

        def k2_group(f2):
            def go():
                wsl = load_wpair(dr["Wk"].ap(), f2, w23, FP8)
                for fh in range(2):
                    f = 2 * f2 + fh
                    aps = accp.tile([P, 512], F32, tag="acc", name="acc")
                    for c2 in range(NCT // 2):
                        nc.tensor.matmul(aps[:, 0:TI],
                                         wsl[:, 2 * c2:2 * c2 + 2,
                                             128 * fh:128 * (fh + 1)],
                                         imgT[c2], start=(c2 == 0),
                                         stop=(c2 == NCT // 2 - 1), perf_mode=DR)
                    nc.vector.tensor_scalar(out=k2_t[f2][:, fh, :],
                                            in0=aps[:, 0:TI],
                                            scalar1=8.0 / FP8_SCALE, scalar2=None,
                                            op0=OP.mult)
            return go

        def v2_group(cc):
            def go():
                whi = load_wpair(dr["Wv"].ap(), cc, w23, FP8)
                for st in range(TI // P):
                    vps = accp.tile([P, 512], F32, tag="acc", name="acc")[:, 0:256]
                    for c2 in range(NCT // 2):
                        nc.tensor.matmul(vps, imgT[c2][:, :, st * P:(st + 1) * P],
                                         whi[:, 2 * c2:2 * c2 + 2, :],
                                         start=(c2 == 0), stop=(c2 == NCT // 2 - 1),
                                         perf_mode=DR)
                    dst = v2aug.rearrange(
                        "p two (h x) -> p two h x", x=65)[:, st, 4 * cc:4 * (cc + 1), 0:64]
                    nc.vector.tensor_scalar(
                        out=dst, in0=vps.rearrange("p (h x) -> p h x", x=64),
                        scalar1=1.0 / FP8_SCALE, scalar2=None, op0=OP.mult)
            return go

        # ---- post-attention pipeline: aproj, ln1b, q2, cross attention ----
        # aproj stays 16-bit: its moving operand (normalized AV) would need an
        # hi/lo split to survive fp8 at the concentrated-attention early
        # tokens, and f16 at 1 cycle/row is cheaper than that.
        def aproj_co2(co2, tch):
            def go():
                sl = slice(512 * tch, 512 * (tch + 1))
                wsl = load_wpair(W2d("W_aproj"), co2, w23, F16)
                for ch in range(2):
                    co = 2 * co2 + ch
                    aps = accp.tile([P, 512], F32, tag="acc", name="acc")
                    for c in range(NCT):
                        nc.tensor.matmul(aps, wsl[:, c, 128 * ch:128 * (ch + 1)],
                                         otiles[c][:, sl],
                                         start=(c == 0), stop=(c == NCT - 1))
                    nc.vector.scalar_tensor_tensor(
                        out=xT[co][:, sl], in0=aps, scalar=bap_c[:, co:co + 1],
                        in1=xT[co][:, sl], op0=OP.add, op1=OP.add)
            return go

        A_b2, B_b2 = ln_ab()

        def ln1b_tch(tch):
            def go():
                ln_stats_tch(xT, psS, tch, A_b2, B_b2)
            return go

        hb_t = [None, None]

        def hb_tch(tch):
            def go():
                hb_t[tch] = ln_apply_pair(
                    A_b2, B_b2, hbp, slice(512 * tch, 512 * (tch + 1)))
            return go

        def q2_group(f2, tch):
            def go():
                wsl = load_wpair(dr["Wq"].ap(), f2, w23, FP8)
                for fh in range(2):
                    f = 2 * f2 + fh
                    aps = accp.tile([P, 512], F32, tag="acc", name="acc")
                    for c2 in range(NCT // 2):
                        nc.tensor.matmul(aps,
                                         wsl[:, 2 * c2:2 * c2 + 2,
                                             128 * fh:128 * (fh + 1)],
                                         hb_t[tch][c2], start=(c2 == 0),
                                         stop=(c2 == NCT // 2 - 1), perf_mode=DR)
                    nc.scalar.activation(
                        out=q2_t[f2][:, fh, 512 * tch:512 * (tch + 1)], in_=aps,
                        func=AF.Identity, bias=bq_c[:, f:f + 1],
                        scale=8.0 / FP8_SCALE)
            return go

        def o_self(h, base):
            return otiles[h // 2][(h % 2) * D:(h % 2) * D + D, base:base + 512]

        # tch0 window: inline qkv for BOTH chunks (PE-dense against the
        # Act-bound softmax); tch1 window: drained cross-attn prep + aproj/
        # ln1b/q2 for chunk 0.
        for tch in range(2):
            if tch == 1:
                side += [k2_group(f2) for f2 in range(4)]
                side += [v2_group(cc) for cc in range(4)]
                side += [aproj_co2(co2, 0) for co2 in range(4)]
                side += [ln1b_tch(0), hb_tch(0)]
                side += [q2_group(f2, 0) for f2 in range(4)]
            for g in range(4):
                if tch == 0:
                    v_group(g)
                    qk_pair(g, g, 2 * g)
                    qk_pair(4 + g, 4 + g, 8 + 2 * g)
                for h in range(4 * g, 4 * g + 4):
                    attn_chunk(kq_self, (va_hi, va_lo), 2 * (tch + 1), o_self,
                               h, tch, psS, pp, causal=True)
                    drain(1)
        drain(len(side))
        qk_cm.__exit__(None, None, None)
        vap_cm.__exit__(None, None, None)

        # ---- cross attention (q2 written post-attention) ----
        psS2 = psS
        pp2 = pp
        # h3 opens before o2 so chunk-0 LN2 work can drain into the
        # cross-attention window (cproj(0) completes mid-window)
        h3_cm, h3p = openp(name="h3", bufs=NCT, side="right")
        o2_cm, opool2 = openp(name="o2", bufs=NCT // 2)
        # o2 c-pair fp8 tiles (x8 scale via the 0.125 ones column) feed the
        # cproj DoubleRow matmuls
        o2tiles = [opool2.tile([P, 2, T], FP8, tag="ot", name="ot")
                   for _ in range(NCT // 2)]

        def o_cross(h, base):
            return o2tiles[h // 4][(h % 2) * D:(h % 2) * D + D, (h // 2) % 2,
                                   base:base + 512]

        cproj_slabs = {}

        def cproj_co2(co2, tch):
            def go():
                sl = slice(512 * tch, 512 * (tch + 1))
                if co2 in cproj_slabs:
                    wsl = cproj_slabs.pop(co2)
                else:
                    wsl = load_wpair(dr["Wcproj"].ap(), co2, w23, FP8)
                if tch == 0:
                    cproj_slabs[co2] = wsl
                for ch in range(2):
                    co = 2 * co2 + ch
                    aps = accp.tile([P, 512], F32, tag="acc", name="acc")
                    for c2 in range(NCT // 2):
                        nc.tensor.matmul(aps,
                                         wsl[:, 2 * c2:2 * c2 + 2,
                                             128 * ch:128 * (ch + 1)],
                                         o2tiles[c2][:, :, sl], start=(c2 == 0),
                                         stop=(c2 == NCT // 2 - 1), perf_mode=DR)
                    # Act adds the bias (and undoes the x64 weight / x8 o2
                    # scaling), gpsimd folds into the residual
                    ctmp = hbp.tile([P, 512], BF16, tag="ctmp", name="ctmp")
                    nc.scalar.activation(out=ctmp, in_=aps, func=AF.Identity,
                                         bias=bcp_c[:, co:co + 1],
                                         scale=1.0 / (8 * FP8_SCALE))
                    nc.gpsimd.tensor_tensor(out=xT[co][:, sl], in0=xT[co][:, sl],
                                            in1=ctmp, op=OP.add)
            return go

        A_b3, B_b3 = ln_ab()
        h2_both = [None, None]

        def ln2_tch(tch):
            def go():
                ln_stats_tch(xT, psS, tch, A_b3, B_b3)
            return go

        def h2split_tch(tch):
            def go():
                h2_both[tch] = ln_apply_split(
                    A_b3, B_b3, h3p, slice(512 * tch, 512 * (tch + 1)))
            return go

        side = [aproj_co2(co2, 1) for co2 in range(4)]
        side += [ln1b_tch(1), hb_tch(1)]
        side += [q2_group(f2, 1) for f2 in range(4)]
        for h in range(H):
            attn_chunk(kq_cross, ([v2aug],), 1, o_cross, h, 0, psS2, pp2,
                       causal=False)
            drain(1)
        drain(len(side))
        side = [cproj_co2(co2, 0) for co2 in range(4)]
        side += [ln2_tch(0), h2split_tch(0)]
        for h in range(H):
            attn_chunk(kq_cross, ([v2aug],), 1, o_cross, h, 1, psS2, pp2,
                       causal=False)
            drain(1)
        drain(len(side))
        for co2 in range(4):
            cproj_co2(co2, 1)()

        o2_cm.__exit__(None, None, None)
        psS_cm.__exit__(None, None, None)
        pp_cm.__exit__(None, None, None)
        o_cm.__exit__(None, None, None)
        psAC_cm.__exit__(None, None, None)
        w23_cm.__exit__(None, None, None)
        q2_cm.__exit__(None, None, None)
        hb_cm.__exit__(None, None, None)
        v2_cm.__exit__(None, None, None)
        k2_cm.__exit__(None, None, None)
        h1_cm.__exit__(None, None, None)
        img_cm.__exit__(None, None, None)

        # ================= P6: MLP =================
        up_cm, up = openp(name="u", bufs=64, side="right")

        def transpose_out(tch):
            tok_cm, tokp = openp(name=f"tok7{tch}", bufs=3)
            tp_cm, tpp = openp(name=f"psT7{tch}", bufs=4, space="PSUM")
            for tt in range(4 * tch, 4 * (tch + 1)):
                otok = tokp.tile([P, C], F32, tag="tok", name="tok")
                for c4 in range(0, NCT, 4):
                    tp4 = tpp.tile([P, 512], F32R, tag="tpr", name="tpr")
                    for i in range(4):
                        nc.tensor.transpose(tp4[:, i * P:(i + 1) * P],
                                            xT[c4 + i][:, tt * P:(tt + 1) * P],
                                            identR)
                    if (tt + c4) % 2:
                        nc.scalar.copy(out=otok[:, c4 * P:(c4 + 4) * P], in_=tp4)
                    else:
                        nc.vector.tensor_copy(out=otok[:, c4 * P:(c4 + 4) * P], in_=tp4)
                nc.sync.dma_start(out=out_d.ap()[tt * P:(tt + 1) * P, :], in_=otok)
            tp_cm.__exit__(None, None, None)
            tok_cm.__exit__(None, None, None)

        # fc with tch as the inner loop: each W_fc slab pair is loaded once
        # and consumed by both token chunks (halves the fc weight traffic)
        u_hi = [[up.tile([P, 2, 512], FP8, tag="u", name="u") for _ in range(16)]
                for _ in range(2)]
        u_lo = [[up.tile([P, 2, 512], FP8, tag="u", name="u") for _ in range(16)]
                for _ in range(2)]
        w5_cm, w5 = openp(name="w5", bufs=4)
        accu_cm, accu = openp(name="psU", bufs=2, space="PSUM")
        # chunk-0 fc for the first slabs runs before the LN2(1) stats so the
        # PE has work while cproj(1) finishes; tch1 for those slabs is
        # re-run at the end (4 extra slab loads)
        fc_sched = [(f2, (0,)) for f2 in range(4)]
        fc_sched += [(f2, (0, 1)) for f2 in range(4, NFT // 2)]
        fc_sched += [(f2, (1,)) for f2 in range(4)]
        done_ln2 = False
        for f2, tchs in fc_sched:
            if f2 == 4 and not done_ln2:
                psL2_cm, psL2 = openp(name="psL2", bufs=1, space="PSUM")
                ln_stats_tch(xT, psL2, 1, A_b3, B_b3, tag="lnst")
                psL2_cm.__exit__(None, None, None)
                h2_both[1] = ln_apply_split(A_b3, B_b3, h3p, slice(512, 1024))
                done_ln2 = True
            whi = load_wpair(dr["W_fc_hi"].ap(), f2, w5, FP8)
            wlo = load_wpair(dr["W_fc_lo"].ap(), f2, w5, FP8)
            for tch in tchs:
                hhi, hlo = h2_both[tch]
                for fh in range(2):
                    ff = 2 * f2 + fh
                    ups = accu.tile([P, 512], F32, tag="acc", name="acc")
                    n = 0
                    # 3-term split: (Hhi+Hlo)@Whi + Hhi@Wlo
                    for wsl, hts in ((whi, hhi), (whi, hlo), (wlo, hhi)):
                        for c2 in range(NCT // 2):
                            nc.tensor.matmul(
                                ups, wsl[:, 2 * c2:2 * c2 + 2, 128 * fh:128 * (fh + 1)],
                                hts[c2], start=(n == 0), stop=(n == 11),
                                perf_mode=DR)
                            n += 1
                    ut = tmpp.tile([P, 512], F16, tag="utmp", name="utmp")
                    nc.scalar.activation(out=ut, in_=ups, func=AF.Gelu_apprx_tanh,
                                         bias=bfc_c[:, ff:ff + 1],
                                         scale=1.0 / FP8_SCALE)
                    nc.vector.tensor_copy(out=u_hi[tch][f2][:, fh, :], in_=ut)
                    e2 = nc.gpsimd if ff % 2 else nc.vector
                    e2.tensor_tensor(out=u_lo[tch][f2][:, fh, :], in0=ut,
                                     in1=u_hi[tch][f2][:, fh, :], op=OP.subtract)
        accu_cm.__exit__(None, None, None)
        w5_cm.__exit__(None, None, None)

        # W_mproj slabs are tch-independent: one slab ring serves both
        # passes, and tch1 walks ff2 DESCENDING so the last three slab pairs
        # of pass 0 are still resident across the transpose_out(0) boundary
        # (no PE stall on slab DMA at the transition).
        w6_cm, w6 = openp(name="w6", bufs=6)
        slab_cache = {}

        def mproj_load(ff2):
            whi = w6.tile([P, 2, C], FP8, tag="mps", name="mps")
            nc.sync.dma_start(
                out=whi, in_=dr["W_mproj_hi"].ap()[ff2 * 256:(ff2 + 1) * 256, :]
                .rearrange("(two p) c -> p two c", p=P))
            wlo = w6.tile([P, 2, C], FP8, tag="mps", name="mps")
            nc.sync.dma_start(
                out=wlo, in_=dr["W_mproj_lo"].ap()[ff2 * 256:(ff2 + 1) * 256, :]
                .rearrange("(two p) c -> p two c", p=P))
            return whi, wlo

        for tch in range(2):
            tsl = slice(512 * tch, 512 * (tch + 1))
            psM_cm, psM = openp(name=f"psM{tch}", bufs=8, space="PSUM")
            mps = [psM.tile([P, 512], F32, tag="m", name="m") for _ in range(NCT)]
            n = 0
            order = range(NFT // 2) if tch == 0 else range(NFT // 2 - 1, -1, -1)
            for ff2 in order:
                if tch == 1 and ff2 in slab_cache:
                    whi, wlo = slab_cache.pop(ff2)
                else:
                    whi, wlo = mproj_load(ff2)
                if tch == 0 and ff2 >= NFT // 2 - 3:
                    slab_cache[ff2] = (whi, wlo)
                for wr, uts in ((whi, u_hi[tch]), (whi, u_lo[tch]),
                                (wlo, u_hi[tch])):
                    for co in range(NCT):
                        nc.tensor.matmul(mps[co], wr[:, :, co * P:(co + 1) * P],
                                         uts[ff2], start=(n == 0),
                                         stop=(n == 3 * (NFT // 2) - 1),
                                         perf_mode=DR)
                    n += 1
            for co in range(NCT):
                nc.vector.scalar_tensor_tensor(
                    out=xT[co][:, tsl], in0=mps[co], scalar=1.0 / FP8_SCALE,
                    in1=xT[co][:, tsl], op0=OP.mult, op1=OP.add)
            psM_cm.__exit__(None, None, None)
            if tch == 0:
                # chunk-0 writeback; its PSUM use fits between the mproj
                # passes, and the copies/DMA overlap mproj(1)'s matmuls
                transpose_out(0)
        w6_cm.__exit__(None, None, None)
        transpose_out(1)

        up_cm.__exit__(None, None, None)
        h3_cm.__exit__(None, None, None)
        xT_cm.__exit__(None, None, None)

        for cm in reversed(kw_cms):
            cm.__exit__(None, None, None)

    nc.compile()
    return nc


def kernel(**inputs):
    import ml_dtypes
    from concourse.bass_utils import run_bass_kernel_spmd

    if "nc" not in _CACHED:
        _CACHED["nc"] = _build()
    nc = _CACHED["nc"]

    f32 = {k: np.asarray(v, dtype=np.float32) for k, v in inputs.items()}
    # Fold LN gains into the consuming weights and LN biases into the
    # consuming projection biases: W^T(xhat*g + b) = (W*g[:,None])^T xhat
    # + W^T b. Exact for any g/b; on-chip LN then only applies (x-mu)*rstd.
    g1, b1v = f32["ln1_g"], f32["ln1_b"]
    g2, b2v = f32["ln2_g"], f32["ln2_b"]
    W_attn, Wq, W_fc = f32["W_attn"], f32["Wq"], f32["W_fc"]
    f32 = dict(f32)
    f32["b_attn"] = f32["b_attn"] + W_attn.T @ b1v
    f32["W_attn"] = W_attn * g1[:, None]
    f32["bq"] = f32["bq"] + Wq.T @ b1v
    f32["Wq"] = Wq * g1[:, None]
    f32["b_fc"] = f32["b_fc"] + W_fc.T @ b2v
    f32["W_fc"] = W_fc * g2[:, None]
    # Bias algebra, all exact:
    # - v/v2 biases commute through the prob-weighted average (sum p = 1),
    #   so they fold into the aproj/cproj output biases.
    # - k biases only shift logits by a per-query constant: softmax-invariant,
    #   dropped. q biases stay (bq . k_s varies per key).
    # - mproj bias is another per-feature residual add: folded into bcproj.
    f32["b_aproj"] = f32["b_aproj"] + f32["b_attn"][2 * C:] @ f32["W_aproj"]
    f32["bcproj"] = (f32["bcproj"] + f32["b_mproj"]
                     + f32["bv"] @ f32["Wcproj"])
    # q/k tiles are stored at x8 (fp8 normal range); bias scales with them
    f32["b_attn"] = f32["b_attn"][:C] * 8.0
    f32["bq"] = f32["bq"] * 8.0

    # column permutation putting head a's d-halves on partitions 32a..32a+31
    # x dim1 (see qk_t layout): new[g*256 + i*128 + a*32 + r] = (4g+a)*64+i*32+r
    perm = np.array([(4 * g + a) * 64 + i * 32 + r
                     for g in range(4) for i in range(2)
                     for a in range(4) for r in range(32)], np.int64)
    W_attn2 = f32["W_attn"].copy()
    W_attn2[:, 0:C] = W_attn2[:, perm]
    W_attn2[:, C:2 * C] = W_attn2[:, C + perm]
    f32["W_attn"] = W_attn2
    f32["b_attn"] = f32["b_attn"][perm]
    f32["Wq"] = f32["Wq"][:, perm]
    f32["bq"] = f32["bq"][perm]
    f32["Wk"] = f32["Wk"][:, perm]

    def fp8_split(w):
        ws = (w * FP8_SCALE).astype(np.float32)
        hi = ws.astype(ml_dtypes.float8_e4m3)
        lo = (ws - hi.astype(np.float32)).astype(ml_dtypes.float8_e4m3)
        return hi, lo

    def fp8_cast(w):
        return (w * FP8_SCALE).astype(ml_dtypes.float8_e4m3)

    f32["W_attn_hi"], f32["W_attn_lo"] = fp8_split(f32.pop("W_attn"))
    f32["W_fc_hi"], f32["W_fc_lo"] = fp8_split(f32.pop("W_fc"))
    f32["W_mproj_hi"], f32["W_mproj_lo"] = fp8_split(f32.pop("W_mproj"))
    for nm in ("Wq", "Wk", "Wv", "Wcproj"):
        f32[nm] = fp8_cast(f32[nm])
    f32["W_aproj"] = f32["W_aproj"].astype(np.float16)
    for nm in ("b_mproj", "bk", "bv"):
        del f32[nm]
    np_inputs = {}
    for k, v in f32.items():
        np_inputs[k] = v.astype(ml_dtypes.bfloat16) if k in WEIGHT_NAMES else v
    in_maps = []
    for b in range(B):
        m = dict(np_inputs)
        m["x"] = np.ascontiguousarray(np_inputs["x"][b])
        m["x_img_feats"] = np.ascontiguousarray(np_inputs["x_img_feats"][b])
        in_maps.append(m)
    res = run_bass_kernel_spmd(nc, in_maps, core_ids=list(range(B)))
    out = np.stack([res.results[b]["out"] for b in range(B)], axis=0)
    return out.astype(np.float32)

